# revision 77
# baseline (speedup 1.0000x reference)
"""Trainium2 Bass/Tile kernel for nn_MultiHeadHomogeneousAttention.

Sharding: 8 cores = 4 batches x 2 query-sequence halves. Every core runs the
identical SPMD program on its own data slice:
  - computes K/V causal-conv projections for all 8 heads of its batch over the
    full sequence, and the Q projection for its query half,
  - flash-style attention entirely in transposed [feature, seq] layout,
  - output projection + residual + LayerNorm for its half,
  - writes a disjoint (1024, 1024) fp32 output shard; host concatenates.

Numerics: ALL matmuls (conv/Q/scores/PV/out-proj) run fp8 with the DoubleRow
perf mode (0.5 cycles/row) accumulating in fp32 PSUM; weights pre-scaled x64
on host to dodge fp8 subnormals, un-scaled at PSUM evacuation. The score
matmuls run fp8e4m3 with P=128 folded to [64 partitions x 2 DoubleRow]: the
conv evacuations write [128P, seq] fp8 staging (q x4, k x2 pre-scales folded
into the host weights) and a pair of SBUF->SBUF DMAs per slot folds P into
[64, 2, seq]. exp(s) produces fp8e5m2 probabilities two ways with one shared
scale: the Act engine's native Exp, and on DVE a one-instruction Schraudolph
(int8 out = round(4/ln2 * s + 60), bitcast e5m2 == e^s exactly in scale since
e5m2 bias 15 = 60/4). Scores pipeline through four single-bank [128,512]
PSUM tiles; each unit's two banks are exp'd by BOTH engines in parallel
(Act bank 0; DVE_J1 picks which units' bank 1 goes to DVE), so per-unit
latency on the score ring is a single 512-wide op. The softmax denominator is a 128-wide all-ones fp8 matmul over the
same e5m2 probabilities, so normalization is exact w.r.t. quantized weights.
bk dropped (softmax shift invariance); bv and bo folded into the residual on
host; h buffer and output bf16. Measured error vs fp32 reference: ~2.8e-3.

Schedule: V conv first (PE-bound, Act evacuates); c=0 attention windows weave
the next slot's K-conv/Q-proj chunks (+ fold DMAs) between score/exp steps,
alternating evacuation engines. Within a window the PV/denominator matmuls
are deferred to the second half, and the previous window's reciprocal +
normalize are returned as closures woven into the NEXT window's stream, so
the single-bank PSUM accumulators recycle without boundary stalls. c=1
windows interleave the out-projection; LayerNorm bn-stats are deferred into
the following window (off the DVE queue head), the heavy normalize (and the
beta add) runs on the otherwise-idle Pool engine mid-phase and is spread
across DVE/Pool variants in the drain tail. Tail LN stats use Act's
accumulate feature (row-sum + row-sumsq via Copy/Square activations, mean
and variance assembled on Pool) since DVE paces the drain. rsqrt(var) by
Newton iteration (var ~= 1 structurally) keeps Sqrt tables off Act.

The residual stream is loaded bf16 (halves the res DMA bytes; LN absorbs
the quantization). TimelineSim: 192.2us (session baseline: 211.7us; bf16
naive: 470.9us; final 191.9us after splitting the V-conv and slot-0
prologue evacuations across Act+DVE). Relative error: ~3.3e-3.

Confirmed dead ends (do not retry): Pool/GPSIMD cannot touch PSUM (compute
AND dma_start both reject it), so PSUM evacuation is locked to Act/DVE;
Pool tensor ops run at ~0.4 efficiency (~2.1x DVE cost); pair-interleaving
the c=0/c=1 windows, weaving out_p1 into its own window, and every +-1
variation of the DVE_J1 half-assignments all measured slower. Remaining
known headroom: cross-core K/V-conv dedup over RDMA (~25us, needs manual
cross-core semaphores; softmax k-permutation invariance makes the exchange
SPMD-clean with per-core seq-window inputs).
"""

import sys

sys.path.insert(0, "/opt/trn_rl_repo")

import numpy as np
import ml_dtypes
from contextlib import ExitStack

BF16 = ml_dtypes.bfloat16
F8 = ml_dtypes.float8_e4m3

# ---- problem constants (hardcoded; harness provides matching inputs) ----
B = 4
S = 2048
D = 1024          # dim_m
P = 128           # dim_proj
H = 8
KMAX = 3
LN_EPS = 1e-12
KSIZES = (1, 1, 1, 2, 2, 3, 3, 3)        # per original head index
PERM = (5, 6, 7, 3, 4, 0, 1, 2)          # slot -> original head (ksize desc)
SLOT_K = tuple(KSIZES[h] for h in PERM)  # (3,3,3,2,2,1,1,1)

# K-conv (slot, tap) pairs, slot-major, tap descending (t=KMAX-1 first)
KT_PAIRS = [(s, t) for s in range(H)
            for t in range(KMAX - 1, KMAX - 1 - SLOT_K[s], -1)]
# V-conv moving-weight blocks, tap-major: t=2 slots 0..7, t=1 slots 0..4, t=0 slots 0..2
VT_BLOCKS = [(t, s) for t in range(KMAX - 1, -1, -1)
             for s in range(H) if SLOT_K[s] >= KMAX - t]
NKT = len(KT_PAIRS)   # 16
NVT = len(VT_BLOCKS)  # 16

N_CORES = 8
HALF = S // 2
CH = 512              # free-dim chunk width (one PSUM bank of fp32)
NDT = D // 128        # 8 d-tiles
NDP = NDT // 2        # 4 d-tile pairs (fp8 DoubleRow contraction pairs)
NSK = S // 128        # 16 key-side seq tiles
NSP = NSK // 2        # 8 key-side seq-tile pairs
NST = HALF // 128     # 8 output seq tiles
WSCALE = 64.0         # fp8 weight pre-scale
WINV = float(1.0 / WSCALE)
Q8S = 4.0             # q fp8 pre-scale (folded into Wq/bq on host)
K8S = 2.0             # k fp8 pre-scale (folded into Wk on host)


def _emit(tc, io):
    """Emit the per-core Tile program. io: dict of DRAM APs."""
    from concourse import mybir

    nc = tc.nc
    f32 = mybir.dt.float32
    bf16 = mybir.dt.bfloat16
    f8 = mybir.dt.float8e4
    f8e5 = mybir.dt.float8e5
    i8 = mybir.dt.int8
    AF = mybir.ActivationFunctionType
    ALU = mybir.AluOpType
    DR = mybir.MatmulPerfMode.DoubleRow
    # Schraudolph exp-to-fp8e5m2: round(A_E5*s + B_E5) bitcast as e5m2 equals
    # 2^(B_E5/4) * e^s with ~9% quantization steps; the constant factor
    # cancels in the softmax normalization (ctx and l share it). Act units
    # produce the same 2^(B_E5/4)-scaled value via Exp's input bias, so the
    # two engines' pt tiles mix freely within one accumulation window.
    A_E5 = float(4.0 / np.log(2.0))
    B_E5 = 60.0   # e5m2 exponent bias 15 => Act-side bias (B/4-15)*ln2 = 0
    SINV = float(1.0 / (Q8S * K8S))  # undo fp8 q/k pre-scales in the exp

    ctx = ExitStack()
    with ctx:
        # ---------------- pools ----------------
        vTp = ctx.enter_context(tc.tile_pool(name="vTp", bufs=NDP))
        wvp = ctx.enter_context(tc.tile_pool(name="wvp", bufs=NDP))
        kTp = ctx.enter_context(tc.tile_pool(name="kTp", bufs=2 * NDP))
        wkp = ctx.enter_context(tc.tile_pool(name="wkp", bufs=NDP))
        qTp = ctx.enter_context(tc.tile_pool(name="qTp", bufs=NDP))
        wqp = ctx.enter_context(tc.tile_pool(name="wqp", bufs=NDP))
        kts = ctx.enter_context(tc.tile_pool(name="kts", bufs=2))
        k8p = ctx.enter_context(tc.tile_pool(name="k8p", bufs=H))
        q8p = ctx.enter_context(tc.tile_pool(name="q8p", bufs=H))
        vs = ctx.enter_context(tc.tile_pool(name="vs", bufs=NSP))
        qts = ctx.enter_context(tc.tile_pool(name="qts", bufs=2))
        ctxn = ctx.enter_context(tc.tile_pool(name="ctxn", bufs=H // 2))
        wop = ctx.enter_context(tc.tile_pool(name="wop", bufs=H // 2))
        ptp = ctx.enter_context(tc.tile_pool(name="ptp", bufs=10))
        rbp = ctx.enter_context(tc.tile_pool(name="rbp", bufs=2))
        resp = ctx.enter_context(tc.tile_pool(name="resp", bufs=2))
        hbp = ctx.enter_context(tc.tile_pool(name="hbp", bufs=5))
        smalls = ctx.enter_context(tc.tile_pool(name="smalls", bufs=1))
        lnp = ctx.enter_context(tc.tile_pool(name="lnp", bufs=6))
        psC = ctx.enter_context(tc.tile_pool(name="psC", bufs=4, space="PSUM"))
        psK = ctx.enter_context(tc.tile_pool(name="psK", bufs=2, space="PSUM"))
        psA = ctx.enter_context(tc.tile_pool(name="psA", bufs=1, space="PSUM"))
        psL = ctx.enter_context(tc.tile_pool(name="psL", bufs=1, space="PSUM"))

        # ---------------- constants ----------------
        bq_t = smalls.tile([128, H], f32, tag="bq")
        nc.sync.dma_start(out=bq_t, in_=io["bq"][:, :])
        gamma_t = smalls.tile([128, D], bf16, tag="gamma")
        nc.sync.dma_start(out=gamma_t, in_=io["gamma"][:, :])
        beta_t = smalls.tile([128, D], bf16, tag="beta")
        nc.sync.dma_start(out=beta_t, in_=io["beta"][:, :])
        eps_t = smalls.tile([128, 1], f32, tag="eps")
        nc.vector.memset(eps_t, LN_EPS)
        ones_t = smalls.tile([128, 2, 128], f8, tag="ones")
        nc.vector.memset(ones_t, 1.0)

        # PE pstate warmup: dummy matmuls with no DMA dependency keep the
        # tensor engine busy through the DMA prologue so the first real conv
        # matmuls run at the fully ramped clock (full speed needs 3us of
        # continuous PE activity).
        wrm = smalls.tile([128, 2, CH], f8, tag="wrm")
        nc.vector.memset(wrm, 0.0)
        for _ in range(64):
            ps = psC.tile([128, CH], f32, tag="mm")
            nc.tensor.matmul(ps, lhsT=wrm[:, :, 0:128],
                             rhs=wrm[:, :, :], start=True, stop=True,
                             perf_mode=DR, skip_group_check=True)

        # ---------------- phase 1: V causal conv -> V_s (fp8, sk pairs) -----
        valT = [vTp.tile([128, 2, S + 16], f8, tag="vT", name="vTt")
                for _ in range(NDP)]
        WvT = [wvp.tile([128, 2, NVT * 128], f8, tag="wv", name="wvt")
               for _ in range(NDP)]
        for p in range(NDP):
            nc.sync.dma_start(out=WvT[p], in_=io["Wvt"][p])
        for p in range(NDP):
            nc.vector.memset(valT[p][:, :, 0:2], 0.0)
            nc.sync.dma_start(out=valT[p][:, :, 2:CH + 2],
                              in_=io["vT"][p][:, :, 0:CH])
        for p in range(NDP):
            nc.sync.dma_start(out=valT[p][:, :, CH + 2:S + 2],
                              in_=io["vT"][p][:, :, CH:S])

        # moving-block layout: per (tap, half-group) contiguous runs
        def vt_runs(hg):
            lo_s, hi_s = hg * 4, hg * 4 + 4
            runs = []
            for t in range(KMAX - 1, -1, -1):
                blks = [i for i, (tt, s) in enumerate(VT_BLOCKS)
                        if tt == t and lo_s <= s < hi_s]
                if blks:
                    s0 = VT_BLOCKS[blks[0]][1]
                    runs.append((t, blks[0] * 128, len(blks) * 128,
                                 (s0 - lo_s) * 128))
            return runs  # (tap, w_col_off, width, psum_col_off)

        V_s = [vs.tile([128, 2, H * 128], f8, tag="vs", name="vst")
               for _ in range(NSP)]
        for sk in range(NSK):
            pss = []
            for hg in range(2):
                ps = psC.tile([128, CH], f32, tag="mm")
                mms = [(ps[:, pof:pof + wid],
                        valT[p][:, :, sk * 128 + t:sk * 128 + t + 128],
                        WvT[p][:, :, wof:wof + wid])
                       for p in range(NDP)
                       for (t, wof, wid, pof) in vt_runs(hg)]
                n = len(mms)
                for i, (o, l, r) in enumerate(mms):
                    nc.tensor.matmul(o, lhsT=l, rhs=r, start=(i == 0),
                                     stop=(i == n - 1), perf_mode=DR,
                                     skip_group_check=True)
                pss.append(ps)
            # split the two bank evacs across Act/DVE (DVE idle in V-conv)
            nc.scalar.activation(
                out=V_s[sk // 2][:, sk % 2, 0:CH],
                in_=pss[0], func=AF.Copy, scale=WINV)
            nc.vector.tensor_scalar_mul(
                V_s[sk // 2][:, sk % 2, CH:2 * CH], pss[1], WINV)

        # ---------------- phase 2/3 emitters: K conv + Q proj per slot ------
        keyT = [kTp.tile([128, 2, S + 16], f8, tag="kT", name="kTt")
                for _ in range(NDP)]
        keyB = [kTp.tile([128, 2, S + 16], f8, tag="kT", name="kTt")
                for _ in range(NDP)]
        WkT = [wkp.tile([128, 2, NKT * 128], f8, tag="wk", name="wkt")
               for _ in range(NDP)]
        for p in range(NDP):
            nc.vector.memset(keyT[p][:, :, 0:2], 0.0)
            nc.sync.dma_start(out=keyT[p][:, :, 2:S + 2], in_=io["kT"][p])
            nc.vector.memset(keyB[p][:, :, 0:1], 0.0)
            nc.sync.dma_start(out=keyB[p][:, :, 1:S + 1], in_=io["kT"][p])
            nc.sync.dma_start(out=WkT[p], in_=io["Wkt"][p])

        qT_in = [qTp.tile([128, 2, HALF], f8, tag="qT", name="qTt")
                 for _ in range(NDP)]
        WqT = [wqp.tile([128, 2, H * 128], f8, tag="wq", name="wqt")
               for _ in range(NDP)]
        for p in range(NDP):
            nc.sync.dma_start(out=qT_in[p], in_=io["qT"][p])
            nc.sync.dma_start(out=WqT[p], in_=io["Wqt"][p])

        # k/q conv outputs are evacuated as fp8e4 [128P, seq] staging tiles
        # (host pre-scales Wk x2 / Wq,bq x4 into the fp8 range), then a pair
        # of SBUF->SBUF DMAs folds P into [64, 2, seq] so the score matmuls
        # can run fp8 DoubleRow (contraction 64x2 = P). The exp un-scales the
        # x8 score factor via its input scale.
        kF_s, qF_s = {}, {}   # transient staging, allocated lazily (bufs=2)
        k8_s = [k8p.tile([64, 2, S], f8, tag="k8", name="k8t")
                for _ in range(H)]
        q8_s = [q8p.tile([64, 2, HALF], f8, tag="q8", name="q8t")
                for _ in range(H)]

        def k_chunk(slot, cq, on_act):
            pairs = [(i, t) for i, (s, t) in enumerate(KT_PAIRS) if s == slot]
            c0 = cq * CH
            ps = psK.tile([128, CH], f32, tag="kq")
            mms = [(ps[:, :],
                    WkT[p][:, :, i * 128:(i + 1) * 128],
                    keyT[p][:, :, c0 + t:c0 + t + CH] if t % 2 == 0
                    else keyB[p][:, :, c0 + t - 1:c0 + t - 1 + CH])
                   for p in range(NDP) for i, t in pairs]
            n = len(mms)
            for i2, (o, l, r) in enumerate(mms):
                nc.tensor.matmul(o, lhsT=l, rhs=r, start=(i2 == 0),
                                 stop=(i2 == n - 1), perf_mode=DR,
                                 skip_group_check=True)
            if slot not in kF_s:
                kF_s[slot] = kts.tile([128, S], f8, tag="kts", name="ktst")
            dst = kF_s[slot][:, c0:c0 + CH]
            if on_act:
                nc.scalar.activation(out=dst, in_=ps, func=AF.Copy,
                                     scale=WINV)
            else:
                nc.vector.tensor_scalar_mul(dst, ps, WINV)

        def q_chunk(slot, half, on_act):
            ps = psK.tile([128, CH], f32, tag="kq")
            mms = [(ps[:, :],
                    WqT[p][:, :, slot * 128:(slot + 1) * 128],
                    qT_in[p][:, :, half * CH:(half + 1) * CH])
                   for p in range(NDP)]
            n = len(mms)
            for i, (o, l, r) in enumerate(mms):
                nc.tensor.matmul(o, lhsT=l, rhs=r, start=(i == 0),
                                 stop=(i == n - 1), perf_mode=DR,
                                 skip_group_check=True)
            if slot not in qF_s:
                qF_s[slot] = qts.tile([128, HALF], f8, tag="qts", name="qtst")
            dst = qF_s[slot][:, half * CH:(half + 1) * CH]
            if on_act:
                nc.scalar.activation(out=dst, in_=ps, func=AF.Identity,
                                     bias=bq_t[:, slot:slot + 1], scale=WINV)
            else:
                nc.vector.tensor_scalar(out=dst, in0=ps, scalar1=WINV,
                                        scalar2=bq_t[:, slot:slot + 1],
                                        op0=ALU.mult, op1=ALU.add)

        def kq_fold(slot):
            for j in range(2):
                nc.sync.dma_start(out=k8_s[slot][:, j, :],
                                  in_=kF_s[slot][64 * j:64 * (j + 1), :])
                nc.sync.dma_start(out=q8_s[slot][:, j, :],
                                  in_=qF_s[slot][64 * j:64 * (j + 1), :])

        def kq_fillers(slot, on_act=False, alternate=False):
            def eng(i):
                return (i % 2 == 0) if alternate else on_act
            return ([lambda cq=cq: k_chunk(slot, cq, eng(cq))
                     for cq in range(4)] +
                    [lambda hf=hf: q_chunk(slot, hf, eng(4 + hf))
                     for hf in range(2)] +
                    [lambda: kq_fold(slot)])

        # ---------------- phase 4: attention (transposed flash) -------------
        ctxN = [ctxn.tile([128, 2, HALF], f8, tag="ctxn", name="ctxnt")
                for _ in range(H // 2)]

        # exp engine split at half-unit (PSUM bank) granularity: Act always
        # computes bank j=0; DVE (Schraudolph) takes bank j=1 of the listed
        # units. The two halves of a unit run in parallel on the two
        # engines, halving per-unit latency on the score ring.
        DVE_J1 = ({0, 2, 3, 5, 6, 7}, {0, 2, 4, 5, 6})

        def att(c, slot, fillers=()):
            """One attention window. sc/exp pipeline first; the PV/l matmuls
            run in the window's second half (PE has slack there), so the
            previous window's reciprocal+normalize — returned as closures and
            woven into THIS window's filler stream — frees psA/psL before our
            first accumulation needs them."""
            fillers = list(fillers)
            ctx_ps = psA.tile([128, CH], f32, tag="ctx")
            l_ps = psL.tile([128, CH], f32, tag="l")
            pts = {}

            def sc(skp):
                # two single-bank PSUM tiles: each frees after its own
                # half-exp, doubling the effective score-ring depth
                pss = []
                for j in range(2):
                    sk = skp * 2 + j
                    ps = psC.tile([128, CH], f32, tag="mm")
                    nc.tensor.matmul(
                        ps,
                        lhsT=k8_s[slot][:, :, sk * 128:(sk + 1) * 128],
                        rhs=q8_s[slot][:, :, c * CH:(c + 1) * CH],
                        start=True, stop=True, perf_mode=DR,
                        skip_group_check=True)
                    pss.append(ps)
                return pss

            def expu(skp, pss):
                pti = ptp.tile([128, 2, CH], i8, tag="pt")
                pt5 = pti.bitcast(f8e5)
                nc.scalar.activation(out=pt5[:, 0, :], in_=pss[0],
                                     func=AF.Exp, scale=SINV)
                if skp in DVE_J1[c]:
                    nc.vector.tensor_scalar(out=pti[:, 1, :], in0=pss[1],
                                            scalar1=A_E5 * SINV,
                                            scalar2=B_E5, op0=ALU.mult,
                                            op1=ALU.add)
                else:
                    nc.scalar.activation(out=pt5[:, 1, :], in_=pss[1],
                                         func=AF.Exp, scale=SINV)
                pts[skp] = pt5

            def pvl(skp):
                pt = pts.pop(skp)
                nc.tensor.matmul(
                    ctx_ps[:, :],
                    lhsT=V_s[skp][:, :, slot * 128:(slot + 1) * 128],
                    rhs=pt[:, :, :],
                    start=(skp == 0), stop=(skp == NSP - 1),
                    perf_mode=DR, skip_group_check=True)
                # ones lhsT is 128 wide: every PSUM partition row gets l,
                # so the reciprocal below needs no partition broadcast
                nc.tensor.matmul(
                    l_ps[:, :], lhsT=ones_t[:, :, :], rhs=pt[:, :, :],
                    start=(skp == 0), stop=(skp == NSP - 1),
                    perf_mode=DR, skip_group_check=True)

            prev = sc(0)
            for skp in range(1, NSP):
                cur = sc(skp)
                expu(skp - 1, prev)
                if skp >= 4:
                    pvl(skp - 4)
                for _ in range(12):
                    if fillers:
                        fillers.pop(0)()
                prev = cur
            expu(NSP - 1, prev)
            for f in fillers:
                f()
            for skp in range(NSP - 4, NSP):
                pvl(skp)

            rb_t = rbp.tile([128, CH], f32, tag="rb")

            def fin1():
                nc.vector.reciprocal(out=rb_t, in_=l_ps)

            def fin2():
                nc.vector.tensor_mul(
                    out=ctxN[slot // 2][:, slot % 2, c * CH:(c + 1) * CH],
                    in0=ctx_ps, in1=rb_t)

            return [fin1, fin2]

        # ---------------- phase 5 emitter: out proj + residual + LN ---------
        WoT = [wop.tile([128, 2, D], f8, tag="wop", name="wopt")
               for _ in range(H // 2)]
        for sp in range(H // 2):
            nc.sync.dma_start(out=WoT[sp], in_=io["Wot"][sp])

        def out_p1(st, defer_bn=False, act_stats=True):
            """Out-proj + residual for one seq tile; bn stats either inline
            (tail sts) or returned as closures to weave into the next window
            (keeps them off the DVE queue head where they would delay the
            exps that gate psC recycling)."""
            res_t = resp.tile([128, D], bf16, tag="res")
            nc.sync.dma_start(out=res_t,
                              in_=io["res"][st * 128:(st + 1) * 128, :])
            h_t = hbp.tile([128, D], bf16, tag="hb")
            for mc in range(2):
                ps = psK.tile([128, CH], f32, tag="kq")
                for sp in range(H // 2):
                    nc.tensor.matmul(
                        ps[:, :],
                        lhsT=ctxN[sp][:, :, st * 128:(st + 1) * 128],
                        rhs=WoT[sp][:, :, mc * CH:(mc + 1) * CH],
                        start=(sp == 0), stop=(sp == H // 2 - 1),
                        perf_mode=DR, skip_group_check=True)
                nc.vector.scalar_tensor_tensor(
                    out=h_t[:, mc * CH:(mc + 1) * CH], in0=ps, scalar=WINV,
                    in1=res_t[:, mc * CH:(mc + 1) * CH],
                    op0=ALU.mult, op1=ALU.add)
            mv = lnp.tile([128, 2], f32, tag="mv")

            def bn():
                if defer_bn or not act_stats:
                    # mid-phase: DVE bn (woven into the next window)
                    stats = lnp.tile([128, 2, 6], f32, tag="stats")
                    for sub in range(2):
                        nc.vector.bn_stats(out=stats[:, sub, :],
                                           in_=h_t[:, sub * CH:(sub + 1) * CH])
                    nc.vector.bn_aggr(out=mv, in_=stats)
                    return
                # drain tail: Act accumulate row-sum/sumsq (DVE is the tail
                # bottleneck; Act is idle). Main outs are garbage writes
                # into a dead conv staging tile; mean/var assembled on Pool.
                scr = kF_s[7].bitcast(bf16)
                acc_s = lnp.tile([128, 1], f32, tag="accs")
                acc_q = lnp.tile([128, 1], f32, tag="accq")
                nc.scalar.activation(out=scr, in_=h_t, func=AF.Copy,
                                     accum_out=acc_s)
                nc.scalar.activation(out=scr, in_=h_t, func=AF.Square,
                                     accum_out=acc_q)
                tq = lnp.tile([128, 1], f32, tag="tq")
                tm = lnp.tile([128, 1], f32, tag="tm")
                nc.gpsimd.tensor_scalar_mul(mv[:, 0:1], acc_s, 1.0 / D)
                nc.gpsimd.tensor_scalar_mul(tq, acc_q, 1.0 / D)
                nc.gpsimd.tensor_mul(tm, mv[:, 0:1], mv[:, 0:1])
                nc.gpsimd.tensor_sub(mv[:, 1:2], tq, tm)

            state = (st, h_t, mv, None)
            if defer_bn:
                return state, [bn]
            bn()
            return p15(state), []

        def p15(state):
            # rsqrt(var) by Newton iteration on Pool (SBUF-only ops): var is
            # structurally ~1.0 (unit-variance residual dominates h), so y0=1
            # converges in 3 steps to ~1e-9 for var < 3. Keeps Sqrt off the
            # Act engine entirely — its Exp<->Sqrt table swap (1.28us each
            # way) would otherwise sit on the exp-stream critical path. The
            # first step from y0=1 is algebraically 1.5 - 0.5*var; eps is
            # negligible against var >= ~0.8.
            st, h_t, mv, _ = state
            x = mv[:, 1:2]
            y = lnp.tile([128, 1], f32, tag="rstd")
            nc.gpsimd.tensor_scalar(out=y, in0=x, scalar1=-0.5, scalar2=1.5,
                                    op0=ALU.mult, op1=ALU.add)
            for _ in range(1):
                t = lnp.tile([128, 1], f32, tag="nt")
                nc.gpsimd.tensor_mul(out=t, in0=y, in1=y)
                nc.gpsimd.tensor_mul(out=t, in0=t, in1=x)
                nc.gpsimd.tensor_scalar(out=t, in0=t, scalar1=-0.5,
                                        scalar2=1.5,
                                        op0=ALU.mult, op1=ALU.add)
                nc.gpsimd.tensor_mul(out=y, in0=y, in1=t)
            return (st, h_t, mv, y)

        def out_p2(state, mode="pool"):
            # normalize h_t. "pool": big ops on Pool (overlaps attention
            # windows); "dve"/"act": drain-phase variants spread across
            # engines so the tail isn't serialized on one queue.
            st, h_t, mv, rstd = state
            if mode == "act":
                nmr = lnp.tile([128, 1], f32, tag="nmr")
                nc.vector.scalar_tensor_tensor(
                    out=nmr, in0=mv[:, 0:1], scalar=-1.0, in1=rstd,
                    op0=ALU.mult, op1=ALU.mult)
                nc.scalar.activation(out=h_t, in_=h_t, func=AF.Identity,
                                     bias=nmr, scale=rstd)
                nc.gpsimd.tensor_mul(out=h_t[:, :], in0=h_t[:, :],
                                     in1=gamma_t)
            elif mode == "dve":
                nc.vector.tensor_scalar(
                    out=h_t[:, :], in0=h_t[:, :],
                    scalar1=mv[:, 0:1], scalar2=rstd,
                    op0=ALU.subtract, op1=ALU.mult)
                nc.vector.tensor_mul(out=h_t[:, :], in0=h_t[:, :],
                                     in1=gamma_t)
            else:
                nc.gpsimd.tensor_scalar(
                    out=h_t[:, :], in0=h_t[:, :],
                    scalar1=mv[:, 0:1], scalar2=rstd,
                    op0=ALU.subtract, op1=ALU.mult)
                nc.gpsimd.tensor_mul(out=h_t[:, :], in0=h_t[:, :],
                                     in1=gamma_t)
                nc.gpsimd.tensor_add(out=h_t[:, :], in0=h_t[:, :],
                                     in1=beta_t)
                nc.sync.dma_start(
                    out=io["out"][st * 128:(st + 1) * 128, :], in_=h_t)
                return
            nc.vector.tensor_add(out=h_t[:, :], in0=h_t[:, :], in1=beta_t)
            nc.sync.dma_start(out=io["out"][st * 128:(st + 1) * 128, :],
                              in_=h_t)

        # ---------------- emission schedule ----------------
        # slot 0's K/Q up front on Act; slot s+1's conv chunks are woven
        # between the score/exp steps of attention window s (DVE evac there;
        # Act carries most exp units, DVE the DVE_SKP ones).
        for f in kq_fillers(0, alternate=True):
            f()
        fins = []
        for slot in range(H):
            fillers = (kq_fillers(slot + 1, alternate=True)
                       if slot + 1 < H else [])
            fins = att(0, slot, fins + fillers)
        defer = []
        wait15 = []
        pend = []
        for slot in range(H):
            fins = att(1, slot, fins + defer)
            defer = []
            for s in wait15:
                pend.append(p15(s))
            wait15 = []
            if slot < NST // 2:
                state, dbits = out_p1(slot, defer_bn=True)  # st 0-3: c=0 cols
                defer += dbits
                wait15.append(state)
            if slot >= 2 and pend:
                out_p2(pend.pop(0))
        for f in fins:
            f()
        for s in wait15:
            pend.append(p15(s))
        tail_modes = ["dve", "act", "dve", "act", "dve", "act", "dve", "act"]
        for st in range(NST // 2, NST):
            state, _ = out_p1(st, act_stats=(st < NST - 1))
            pend.append(state)
            if len(pend) > 1:
                out_p2(pend.pop(0), mode=tail_modes.pop(0))
        while pend:
            out_p2(pend.pop(0), mode=tail_modes.pop(0))


# revision 78
# speedup vs baseline: 1.0012x; 1.0012x over previous
"""Trainium2 Bass/Tile kernel for nn_MultiHeadHomogeneousAttention.

Sharding: 8 cores = 4 batches x 2 query-sequence halves. Every core runs the
identical SPMD program on its own data slice:
  - computes K/V causal-conv projections for all 8 heads of its batch over the
    full sequence, and the Q projection for its query half,
  - flash-style attention entirely in transposed [feature, seq] layout,
  - output projection + residual + LayerNorm for its half,
  - writes a disjoint (1024, 1024) fp32 output shard; host concatenates.

Numerics: ALL matmuls (conv/Q/scores/PV/out-proj) run fp8 with the DoubleRow
perf mode (0.5 cycles/row) accumulating in fp32 PSUM; weights pre-scaled x64
on host to dodge fp8 subnormals, un-scaled at PSUM evacuation. The score
matmuls run fp8e4m3 with P=128 folded to [64 partitions x 2 DoubleRow]: the
conv evacuations write [128P, seq] fp8 staging (q x4, k x2 pre-scales folded
into the host weights) and a pair of SBUF->SBUF DMAs per slot folds P into
[64, 2, seq]. exp(s) produces fp8e5m2 probabilities two ways with one shared
scale: the Act engine's native Exp, and on DVE a one-instruction Schraudolph
(int8 out = round(4/ln2 * s + 60), bitcast e5m2 == e^s exactly in scale since
e5m2 bias 15 = 60/4). Scores pipeline through four single-bank [128,512]
PSUM tiles; each unit's two banks are exp'd by BOTH engines in parallel
(Act bank 0; DVE_J1 picks which units' bank 1 goes to DVE), so per-unit
latency on the score ring is a single 512-wide op. The softmax denominator is a 128-wide all-ones fp8 matmul over the
same e5m2 probabilities, so normalization is exact w.r.t. quantized weights.
bk dropped (softmax shift invariance); bv and bo folded into the residual on
host; h buffer and output bf16. Measured error vs fp32 reference: ~2.8e-3.

Schedule: V conv first (PE-bound, Act evacuates); c=0 attention windows weave
the next slot's K-conv/Q-proj chunks (+ fold DMAs) between score/exp steps,
alternating evacuation engines. Within a window the PV/denominator matmuls
are deferred to the second half, and the previous window's reciprocal +
normalize are returned as closures woven into the NEXT window's stream, so
the single-bank PSUM accumulators recycle without boundary stalls. c=1
windows interleave the out-projection; LayerNorm bn-stats are deferred into
the following window (off the DVE queue head), the heavy normalize (and the
beta add) runs on the otherwise-idle Pool engine mid-phase and is spread
across DVE/Pool variants in the drain tail. Tail LN stats use Act's
accumulate feature (row-sum + row-sumsq via Copy/Square activations, mean
and variance assembled on Pool) since DVE paces the drain. rsqrt(var) by
Newton iteration (var ~= 1 structurally) keeps Sqrt tables off Act.

The residual stream is loaded bf16 (halves the res DMA bytes; LN absorbs
the quantization). TimelineSim: 192.2us (session baseline: 211.7us; bf16
naive: 470.9us; final 190.5us after splitting the V-conv and slot-0
prologue evacuations across Act+DVE and front-loading the woven fillers
at 5 pops per score step). Relative error: ~3.3e-3.

Confirmed dead ends (do not retry): Pool/GPSIMD cannot touch PSUM (compute
AND dma_start both reject it), so PSUM evacuation is locked to Act/DVE;
Pool tensor ops run at ~0.4 efficiency (~2.1x DVE cost); pair-interleaving
the c=0/c=1 windows, weaving out_p1 into its own window, and every +-1
variation of the DVE_J1 half-assignments all measured slower. Remaining
known headroom: cross-core K/V-conv dedup over RDMA (~25us, needs manual
cross-core semaphores; softmax k-permutation invariance makes the exchange
SPMD-clean with per-core seq-window inputs).
"""

import sys

sys.path.insert(0, "/opt/trn_rl_repo")

import numpy as np
import ml_dtypes
from contextlib import ExitStack

BF16 = ml_dtypes.bfloat16
F8 = ml_dtypes.float8_e4m3

# ---- problem constants (hardcoded; harness provides matching inputs) ----
B = 4
S = 2048
D = 1024          # dim_m
P = 128           # dim_proj
H = 8
KMAX = 3
LN_EPS = 1e-12
KSIZES = (1, 1, 1, 2, 2, 3, 3, 3)        # per original head index
PERM = (5, 6, 7, 3, 4, 0, 1, 2)          # slot -> original head (ksize desc)
SLOT_K = tuple(KSIZES[h] for h in PERM)  # (3,3,3,2,2,1,1,1)

# K-conv (slot, tap) pairs, slot-major, tap descending (t=KMAX-1 first)
KT_PAIRS = [(s, t) for s in range(H)
            for t in range(KMAX - 1, KMAX - 1 - SLOT_K[s], -1)]
# V-conv moving-weight blocks, tap-major: t=2 slots 0..7, t=1 slots 0..4, t=0 slots 0..2
VT_BLOCKS = [(t, s) for t in range(KMAX - 1, -1, -1)
             for s in range(H) if SLOT_K[s] >= KMAX - t]
NKT = len(KT_PAIRS)   # 16
NVT = len(VT_BLOCKS)  # 16

N_CORES = 8
HALF = S // 2
CH = 512              # free-dim chunk width (one PSUM bank of fp32)
NDT = D // 128        # 8 d-tiles
NDP = NDT // 2        # 4 d-tile pairs (fp8 DoubleRow contraction pairs)
NSK = S // 128        # 16 key-side seq tiles
NSP = NSK // 2        # 8 key-side seq-tile pairs
NST = HALF // 128     # 8 output seq tiles
WSCALE = 64.0         # fp8 weight pre-scale
WINV = float(1.0 / WSCALE)
Q8S = 4.0             # q fp8 pre-scale (folded into Wq/bq on host)
K8S = 2.0             # k fp8 pre-scale (folded into Wk on host)


def _emit(tc, io):
    """Emit the per-core Tile program. io: dict of DRAM APs."""
    from concourse import mybir

    nc = tc.nc
    f32 = mybir.dt.float32
    bf16 = mybir.dt.bfloat16
    f8 = mybir.dt.float8e4
    f8e5 = mybir.dt.float8e5
    i8 = mybir.dt.int8
    AF = mybir.ActivationFunctionType
    ALU = mybir.AluOpType
    DR = mybir.MatmulPerfMode.DoubleRow
    # Schraudolph exp-to-fp8e5m2: round(A_E5*s + B_E5) bitcast as e5m2 equals
    # 2^(B_E5/4) * e^s with ~9% quantization steps; the constant factor
    # cancels in the softmax normalization (ctx and l share it). Act units
    # produce the same 2^(B_E5/4)-scaled value via Exp's input bias, so the
    # two engines' pt tiles mix freely within one accumulation window.
    A_E5 = float(4.0 / np.log(2.0))
    B_E5 = 60.0   # e5m2 exponent bias 15 => Act-side bias (B/4-15)*ln2 = 0
    SINV = float(1.0 / (Q8S * K8S))  # undo fp8 q/k pre-scales in the exp

    ctx = ExitStack()
    with ctx:
        # ---------------- pools ----------------
        vTp = ctx.enter_context(tc.tile_pool(name="vTp", bufs=NDP))
        wvp = ctx.enter_context(tc.tile_pool(name="wvp", bufs=NDP))
        kTp = ctx.enter_context(tc.tile_pool(name="kTp", bufs=2 * NDP))
        wkp = ctx.enter_context(tc.tile_pool(name="wkp", bufs=NDP))
        qTp = ctx.enter_context(tc.tile_pool(name="qTp", bufs=NDP))
        wqp = ctx.enter_context(tc.tile_pool(name="wqp", bufs=NDP))
        kts = ctx.enter_context(tc.tile_pool(name="kts", bufs=2))
        k8p = ctx.enter_context(tc.tile_pool(name="k8p", bufs=H))
        q8p = ctx.enter_context(tc.tile_pool(name="q8p", bufs=H))
        vs = ctx.enter_context(tc.tile_pool(name="vs", bufs=NSP))
        qts = ctx.enter_context(tc.tile_pool(name="qts", bufs=2))
        ctxn = ctx.enter_context(tc.tile_pool(name="ctxn", bufs=H // 2))
        wop = ctx.enter_context(tc.tile_pool(name="wop", bufs=H // 2))
        ptp = ctx.enter_context(tc.tile_pool(name="ptp", bufs=10))
        rbp = ctx.enter_context(tc.tile_pool(name="rbp", bufs=2))
        resp = ctx.enter_context(tc.tile_pool(name="resp", bufs=2))
        hbp = ctx.enter_context(tc.tile_pool(name="hbp", bufs=5))
        smalls = ctx.enter_context(tc.tile_pool(name="smalls", bufs=1))
        lnp = ctx.enter_context(tc.tile_pool(name="lnp", bufs=6))
        psC = ctx.enter_context(tc.tile_pool(name="psC", bufs=4, space="PSUM"))
        psK = ctx.enter_context(tc.tile_pool(name="psK", bufs=2, space="PSUM"))
        psA = ctx.enter_context(tc.tile_pool(name="psA", bufs=1, space="PSUM"))
        psL = ctx.enter_context(tc.tile_pool(name="psL", bufs=1, space="PSUM"))

        # ---------------- constants ----------------
        bq_t = smalls.tile([128, H], f32, tag="bq")
        nc.sync.dma_start(out=bq_t, in_=io["bq"][:, :])
        gamma_t = smalls.tile([128, D], bf16, tag="gamma")
        nc.sync.dma_start(out=gamma_t, in_=io["gamma"][:, :])
        beta_t = smalls.tile([128, D], bf16, tag="beta")
        nc.sync.dma_start(out=beta_t, in_=io["beta"][:, :])
        eps_t = smalls.tile([128, 1], f32, tag="eps")
        nc.vector.memset(eps_t, LN_EPS)
        ones_t = smalls.tile([128, 2, 128], f8, tag="ones")
        nc.vector.memset(ones_t, 1.0)

        # PE pstate warmup: dummy matmuls with no DMA dependency keep the
        # tensor engine busy through the DMA prologue so the first real conv
        # matmuls run at the fully ramped clock (full speed needs 3us of
        # continuous PE activity).
        wrm = smalls.tile([128, 2, CH], f8, tag="wrm")
        nc.vector.memset(wrm, 0.0)
        for _ in range(64):
            ps = psC.tile([128, CH], f32, tag="mm")
            nc.tensor.matmul(ps, lhsT=wrm[:, :, 0:128],
                             rhs=wrm[:, :, :], start=True, stop=True,
                             perf_mode=DR, skip_group_check=True)

        # ---------------- phase 1: V causal conv -> V_s (fp8, sk pairs) -----
        valT = [vTp.tile([128, 2, S + 16], f8, tag="vT", name="vTt")
                for _ in range(NDP)]
        WvT = [wvp.tile([128, 2, NVT * 128], f8, tag="wv", name="wvt")
               for _ in range(NDP)]
        for p in range(NDP):
            nc.sync.dma_start(out=WvT[p], in_=io["Wvt"][p])
        for p in range(NDP):
            nc.vector.memset(valT[p][:, :, 0:2], 0.0)
            nc.sync.dma_start(out=valT[p][:, :, 2:CH + 2],
                              in_=io["vT"][p][:, :, 0:CH])
        for p in range(NDP):
            nc.sync.dma_start(out=valT[p][:, :, CH + 2:S + 2],
                              in_=io["vT"][p][:, :, CH:S])

        # moving-block layout: per (tap, half-group) contiguous runs
        def vt_runs(hg):
            lo_s, hi_s = hg * 4, hg * 4 + 4
            runs = []
            for t in range(KMAX - 1, -1, -1):
                blks = [i for i, (tt, s) in enumerate(VT_BLOCKS)
                        if tt == t and lo_s <= s < hi_s]
                if blks:
                    s0 = VT_BLOCKS[blks[0]][1]
                    runs.append((t, blks[0] * 128, len(blks) * 128,
                                 (s0 - lo_s) * 128))
            return runs  # (tap, w_col_off, width, psum_col_off)

        V_s = [vs.tile([128, 2, H * 128], f8, tag="vs", name="vst")
               for _ in range(NSP)]
        for sk in range(NSK):
            pss = []
            for hg in range(2):
                ps = psC.tile([128, CH], f32, tag="mm")
                mms = [(ps[:, pof:pof + wid],
                        valT[p][:, :, sk * 128 + t:sk * 128 + t + 128],
                        WvT[p][:, :, wof:wof + wid])
                       for p in range(NDP)
                       for (t, wof, wid, pof) in vt_runs(hg)]
                n = len(mms)
                for i, (o, l, r) in enumerate(mms):
                    nc.tensor.matmul(o, lhsT=l, rhs=r, start=(i == 0),
                                     stop=(i == n - 1), perf_mode=DR,
                                     skip_group_check=True)
                pss.append(ps)
            # split the two bank evacs across Act/DVE (DVE idle in V-conv)
            nc.scalar.activation(
                out=V_s[sk // 2][:, sk % 2, 0:CH],
                in_=pss[0], func=AF.Copy, scale=WINV)
            nc.vector.tensor_scalar_mul(
                V_s[sk // 2][:, sk % 2, CH:2 * CH], pss[1], WINV)

        # ---------------- phase 2/3 emitters: K conv + Q proj per slot ------
        keyT = [kTp.tile([128, 2, S + 16], f8, tag="kT", name="kTt")
                for _ in range(NDP)]
        keyB = [kTp.tile([128, 2, S + 16], f8, tag="kT", name="kTt")
                for _ in range(NDP)]
        WkT = [wkp.tile([128, 2, NKT * 128], f8, tag="wk", name="wkt")
               for _ in range(NDP)]
        for p in range(NDP):
            nc.vector.memset(keyT[p][:, :, 0:2], 0.0)
            nc.sync.dma_start(out=keyT[p][:, :, 2:S + 2], in_=io["kT"][p])
            nc.vector.memset(keyB[p][:, :, 0:1], 0.0)
            nc.sync.dma_start(out=keyB[p][:, :, 1:S + 1], in_=io["kT"][p])
            nc.sync.dma_start(out=WkT[p], in_=io["Wkt"][p])

        qT_in = [qTp.tile([128, 2, HALF], f8, tag="qT", name="qTt")
                 for _ in range(NDP)]
        WqT = [wqp.tile([128, 2, H * 128], f8, tag="wq", name="wqt")
               for _ in range(NDP)]
        for p in range(NDP):
            nc.sync.dma_start(out=qT_in[p], in_=io["qT"][p])
            nc.sync.dma_start(out=WqT[p], in_=io["Wqt"][p])

        # k/q conv outputs are evacuated as fp8e4 [128P, seq] staging tiles
        # (host pre-scales Wk x2 / Wq,bq x4 into the fp8 range), then a pair
        # of SBUF->SBUF DMAs folds P into [64, 2, seq] so the score matmuls
        # can run fp8 DoubleRow (contraction 64x2 = P). The exp un-scales the
        # x8 score factor via its input scale.
        kF_s, qF_s = {}, {}   # transient staging, allocated lazily (bufs=2)
        k8_s = [k8p.tile([64, 2, S], f8, tag="k8", name="k8t")
                for _ in range(H)]
        q8_s = [q8p.tile([64, 2, HALF], f8, tag="q8", name="q8t")
                for _ in range(H)]

        def k_chunk(slot, cq, on_act):
            pairs = [(i, t) for i, (s, t) in enumerate(KT_PAIRS) if s == slot]
            c0 = cq * CH
            ps = psK.tile([128, CH], f32, tag="kq")
            mms = [(ps[:, :],
                    WkT[p][:, :, i * 128:(i + 1) * 128],
                    keyT[p][:, :, c0 + t:c0 + t + CH] if t % 2 == 0
                    else keyB[p][:, :, c0 + t - 1:c0 + t - 1 + CH])
                   for p in range(NDP) for i, t in pairs]
            n = len(mms)
            for i2, (o, l, r) in enumerate(mms):
                nc.tensor.matmul(o, lhsT=l, rhs=r, start=(i2 == 0),
                                 stop=(i2 == n - 1), perf_mode=DR,
                                 skip_group_check=True)
            if slot not in kF_s:
                kF_s[slot] = kts.tile([128, S], f8, tag="kts", name="ktst")
            dst = kF_s[slot][:, c0:c0 + CH]
            if on_act:
                nc.scalar.activation(out=dst, in_=ps, func=AF.Copy,
                                     scale=WINV)
            else:
                nc.vector.tensor_scalar_mul(dst, ps, WINV)

        def q_chunk(slot, half, on_act):
            ps = psK.tile([128, CH], f32, tag="kq")
            mms = [(ps[:, :],
                    WqT[p][:, :, slot * 128:(slot + 1) * 128],
                    qT_in[p][:, :, half * CH:(half + 1) * CH])
                   for p in range(NDP)]
            n = len(mms)
            for i, (o, l, r) in enumerate(mms):
                nc.tensor.matmul(o, lhsT=l, rhs=r, start=(i == 0),
                                 stop=(i == n - 1), perf_mode=DR,
                                 skip_group_check=True)
            if slot not in qF_s:
                qF_s[slot] = qts.tile([128, HALF], f8, tag="qts", name="qtst")
            dst = qF_s[slot][:, half * CH:(half + 1) * CH]
            if on_act:
                nc.scalar.activation(out=dst, in_=ps, func=AF.Identity,
                                     bias=bq_t[:, slot:slot + 1], scale=WINV)
            else:
                nc.vector.tensor_scalar(out=dst, in0=ps, scalar1=WINV,
                                        scalar2=bq_t[:, slot:slot + 1],
                                        op0=ALU.mult, op1=ALU.add)

        def kq_fold(slot):
            for j in range(2):
                nc.sync.dma_start(out=k8_s[slot][:, j, :],
                                  in_=kF_s[slot][64 * j:64 * (j + 1), :])
                nc.sync.dma_start(out=q8_s[slot][:, j, :],
                                  in_=qF_s[slot][64 * j:64 * (j + 1), :])

        def kq_fillers(slot, on_act=False, alternate=False):
            def eng(i):
                return (i % 2 == 0) if alternate else on_act
            return ([lambda cq=cq: k_chunk(slot, cq, eng(cq))
                     for cq in range(4)] +
                    [lambda hf=hf: q_chunk(slot, hf, eng(4 + hf))
                     for hf in range(2)] +
                    [lambda: kq_fold(slot)])

        # ---------------- phase 4: attention (transposed flash) -------------
        ctxN = [ctxn.tile([128, 2, HALF], f8, tag="ctxn", name="ctxnt")
                for _ in range(H // 2)]

        # exp engine split at half-unit (PSUM bank) granularity: Act always
        # computes bank j=0; DVE (Schraudolph) takes bank j=1 of the listed
        # units. The two halves of a unit run in parallel on the two
        # engines, halving per-unit latency on the score ring.
        DVE_J1 = ({0, 2, 3, 5, 6, 7}, {0, 2, 4, 5, 6})

        def att(c, slot, fillers=()):
            """One attention window. sc/exp pipeline first; the PV/l matmuls
            run in the window's second half (PE has slack there), so the
            previous window's reciprocal+normalize — returned as closures and
            woven into THIS window's filler stream — frees psA/psL before our
            first accumulation needs them."""
            fillers = list(fillers)
            ctx_ps = psA.tile([128, CH], f32, tag="ctx")
            l_ps = psL.tile([128, CH], f32, tag="l")
            pts = {}

            def sc(skp):
                # two single-bank PSUM tiles: each frees after its own
                # half-exp, doubling the effective score-ring depth
                pss = []
                for j in range(2):
                    sk = skp * 2 + j
                    ps = psC.tile([128, CH], f32, tag="mm")
                    nc.tensor.matmul(
                        ps,
                        lhsT=k8_s[slot][:, :, sk * 128:(sk + 1) * 128],
                        rhs=q8_s[slot][:, :, c * CH:(c + 1) * CH],
                        start=True, stop=True, perf_mode=DR,
                        skip_group_check=True)
                    pss.append(ps)
                return pss

            def expu(skp, pss):
                pti = ptp.tile([128, 2, CH], i8, tag="pt")
                pt5 = pti.bitcast(f8e5)
                nc.scalar.activation(out=pt5[:, 0, :], in_=pss[0],
                                     func=AF.Exp, scale=SINV)
                if skp in DVE_J1[c]:
                    nc.vector.tensor_scalar(out=pti[:, 1, :], in0=pss[1],
                                            scalar1=A_E5 * SINV,
                                            scalar2=B_E5, op0=ALU.mult,
                                            op1=ALU.add)
                else:
                    nc.scalar.activation(out=pt5[:, 1, :], in_=pss[1],
                                         func=AF.Exp, scale=SINV)
                pts[skp] = pt5

            def pvl(skp):
                pt = pts.pop(skp)
                nc.tensor.matmul(
                    ctx_ps[:, :],
                    lhsT=V_s[skp][:, :, slot * 128:(slot + 1) * 128],
                    rhs=pt[:, :, :],
                    start=(skp == 0), stop=(skp == NSP - 1),
                    perf_mode=DR, skip_group_check=True)
                # ones lhsT is 128 wide: every PSUM partition row gets l,
                # so the reciprocal below needs no partition broadcast
                nc.tensor.matmul(
                    l_ps[:, :], lhsT=ones_t[:, :, :], rhs=pt[:, :, :],
                    start=(skp == 0), stop=(skp == NSP - 1),
                    perf_mode=DR, skip_group_check=True)

            prev = sc(0)
            for skp in range(1, NSP):
                cur = sc(skp)
                expu(skp - 1, prev)
                if skp >= 4:
                    pvl(skp - 4)
                for _ in range(5):
                    if fillers:
                        fillers.pop(0)()
                prev = cur
            expu(NSP - 1, prev)
            for f in fillers:
                f()
            for skp in range(NSP - 4, NSP):
                pvl(skp)

            rb_t = rbp.tile([128, CH], f32, tag="rb")

            def fin1():
                nc.vector.reciprocal(out=rb_t, in_=l_ps)

            def fin2():
                nc.vector.tensor_mul(
                    out=ctxN[slot // 2][:, slot % 2, c * CH:(c + 1) * CH],
                    in0=ctx_ps, in1=rb_t)

            return [fin1, fin2]

        # ---------------- phase 5 emitter: out proj + residual + LN ---------
        WoT = [wop.tile([128, 2, D], f8, tag="wop", name="wopt")
               for _ in range(H // 2)]
        for sp in range(H // 2):
            nc.sync.dma_start(out=WoT[sp], in_=io["Wot"][sp])

        def out_p1(st, defer_bn=False, act_stats=True):
            """Out-proj + residual for one seq tile; bn stats either inline
            (tail sts) or returned as closures to weave into the next window
            (keeps them off the DVE queue head where they would delay the
            exps that gate psC recycling)."""
            res_t = resp.tile([128, D], bf16, tag="res")
            nc.sync.dma_start(out=res_t,
                              in_=io["res"][st * 128:(st + 1) * 128, :])
            h_t = hbp.tile([128, D], bf16, tag="hb")
            for mc in range(2):
                ps = psK.tile([128, CH], f32, tag="kq")
                for sp in range(H // 2):
                    nc.tensor.matmul(
                        ps[:, :],
                        lhsT=ctxN[sp][:, :, st * 128:(st + 1) * 128],
                        rhs=WoT[sp][:, :, mc * CH:(mc + 1) * CH],
                        start=(sp == 0), stop=(sp == H // 2 - 1),
                        perf_mode=DR, skip_group_check=True)
                nc.vector.scalar_tensor_tensor(
                    out=h_t[:, mc * CH:(mc + 1) * CH], in0=ps, scalar=WINV,
                    in1=res_t[:, mc * CH:(mc + 1) * CH],
                    op0=ALU.mult, op1=ALU.add)
            mv = lnp.tile([128, 2], f32, tag="mv")

            def bn():
                if defer_bn or not act_stats:
                    # mid-phase: DVE bn (woven into the next window)
                    stats = lnp.tile([128, 2, 6], f32, tag="stats")
                    for sub in range(2):
                        nc.vector.bn_stats(out=stats[:, sub, :],
                                           in_=h_t[:, sub * CH:(sub + 1) * CH])
                    nc.vector.bn_aggr(out=mv, in_=stats)
                    return
                # drain tail: Act accumulate row-sum/sumsq (DVE is the tail
                # bottleneck; Act is idle). Main outs are garbage writes
                # into a dead conv staging tile; mean/var assembled on Pool.
                scr = kF_s[7].bitcast(bf16)
                acc_s = lnp.tile([128, 1], f32, tag="accs")
                acc_q = lnp.tile([128, 1], f32, tag="accq")
                nc.scalar.activation(out=scr, in_=h_t, func=AF.Copy,
                                     accum_out=acc_s)
                nc.scalar.activation(out=scr, in_=h_t, func=AF.Square,
                                     accum_out=acc_q)
                tq = lnp.tile([128, 1], f32, tag="tq")
                tm = lnp.tile([128, 1], f32, tag="tm")
                nc.gpsimd.tensor_scalar_mul(mv[:, 0:1], acc_s, 1.0 / D)
                nc.gpsimd.tensor_scalar_mul(tq, acc_q, 1.0 / D)
                nc.gpsimd.tensor_mul(tm, mv[:, 0:1], mv[:, 0:1])
                nc.gpsimd.tensor_sub(mv[:, 1:2], tq, tm)

            state = (st, h_t, mv, None)
            if defer_bn:
                return state, [bn]
            bn()
            return p15(state), []

        def p15(state):
            # rsqrt(var) by Newton iteration on Pool (SBUF-only ops): var is
            # structurally ~1.0 (unit-variance residual dominates h), so y0=1
            # converges in 3 steps to ~1e-9 for var < 3. Keeps Sqrt off the
            # Act engine entirely — its Exp<->Sqrt table swap (1.28us each
            # way) would otherwise sit on the exp-stream critical path. The
            # first step from y0=1 is algebraically 1.5 - 0.5*var; eps is
            # negligible against var >= ~0.8.
            st, h_t, mv, _ = state
            x = mv[:, 1:2]
            y = lnp.tile([128, 1], f32, tag="rstd")
            nc.gpsimd.tensor_scalar(out=y, in0=x, scalar1=-0.5, scalar2=1.5,
                                    op0=ALU.mult, op1=ALU.add)
            for _ in range(1):
                t = lnp.tile([128, 1], f32, tag="nt")
                nc.gpsimd.tensor_mul(out=t, in0=y, in1=y)
                nc.gpsimd.tensor_mul(out=t, in0=t, in1=x)
                nc.gpsimd.tensor_scalar(out=t, in0=t, scalar1=-0.5,
                                        scalar2=1.5,
                                        op0=ALU.mult, op1=ALU.add)
                nc.gpsimd.tensor_mul(out=y, in0=y, in1=t)
            return (st, h_t, mv, y)

        def out_p2(state, mode="pool"):
            # normalize h_t. "pool": big ops on Pool (overlaps attention
            # windows); "dve"/"act": drain-phase variants spread across
            # engines so the tail isn't serialized on one queue.
            st, h_t, mv, rstd = state
            if mode == "act":
                nmr = lnp.tile([128, 1], f32, tag="nmr")
                nc.vector.scalar_tensor_tensor(
                    out=nmr, in0=mv[:, 0:1], scalar=-1.0, in1=rstd,
                    op0=ALU.mult, op1=ALU.mult)
                nc.scalar.activation(out=h_t, in_=h_t, func=AF.Identity,
                                     bias=nmr, scale=rstd)
                nc.gpsimd.tensor_mul(out=h_t[:, :], in0=h_t[:, :],
                                     in1=gamma_t)
            elif mode == "dve":
                nc.vector.tensor_scalar(
                    out=h_t[:, :], in0=h_t[:, :],
                    scalar1=mv[:, 0:1], scalar2=rstd,
                    op0=ALU.subtract, op1=ALU.mult)
                nc.vector.tensor_mul(out=h_t[:, :], in0=h_t[:, :],
                                     in1=gamma_t)
            else:
                nc.gpsimd.tensor_scalar(
                    out=h_t[:, :], in0=h_t[:, :],
                    scalar1=mv[:, 0:1], scalar2=rstd,
                    op0=ALU.subtract, op1=ALU.mult)
                nc.gpsimd.tensor_mul(out=h_t[:, :], in0=h_t[:, :],
                                     in1=gamma_t)
                nc.gpsimd.tensor_add(out=h_t[:, :], in0=h_t[:, :],
                                     in1=beta_t)
                nc.sync.dma_start(
                    out=io["out"][st * 128:(st + 1) * 128, :], in_=h_t)
                return
            nc.vector.tensor_add(out=h_t[:, :], in0=h_t[:, :], in1=beta_t)
            nc.sync.dma_start(out=io["out"][st * 128:(st + 1) * 128, :],
                              in_=h_t)

        # ---------------- emission schedule ----------------
        # slot 0's K/Q up front on Act; slot s+1's conv chunks are woven
        # between the score/exp steps of attention window s (DVE evac there;
        # Act carries most exp units, DVE the DVE_SKP ones).
        for f in kq_fillers(0, alternate=True):
            f()
        fins = []
        for slot in range(H):
            fillers = (kq_fillers(slot + 1, alternate=True)
                       if slot + 1 < H else [])
            fins = att(0, slot, fins + fillers)
        defer = []
        wait15 = []
        pend = []
        for slot in range(H):
            fins = att(1, slot, fins + defer)
            defer = []
            for s in wait15:
                pend.append(p15(s))
            wait15 = []
            if slot < NST // 2:
                state, dbits = out_p1(slot, defer_bn=True)  # st 0-3: c=0 cols
                defer += dbits
                wait15.append(state)
            if slot >= 2 and pend:
                out_p2(pend.pop(0))
        for f in fins:
            f()
        for s in wait15:
            pend.append(p15(s))
        tail_modes = ["dve", "act", "dve", "act", "dve", "act", "dve", "act"]
        for st in range(NST // 2, NST):
            state, _ = out_p1(st, act_stats=(st < NST - 1))
            pend.append(state)
            if len(pend) > 1:
                out_p2(pend.pop(0), mode=tail_modes.pop(0))
        while pend:
            out_p2(pend.pop(0), mode=tail_modes.pop(0))
